# revision 43
# baseline (speedup 1.0000x reference)
"""Multi-head causal attention with RoPE on 8 Trainium2 NeuronCores.

Sharding: data-parallel over batch (2 groups of 4 cores) x tensor-parallel
over heads (4 heads / 512 cols of Wq/Wk/Wv per core, 512 rows of Wo).
Each core computes its head-group's Q/K/V projections in transposed layout
([head_dim, seq] -- so no on-device transposes are ever needed), applies
RoPE, runs causal softmax attention (scores kept transposed [tk, tq]), and
emits its partial output projection (fp16).  The host sums the 4 partials
per batch element in fp32.

Schedule (per tq-chunk c >= 1): Q projection -> off-diagonal attention
blocks of heads 0-2 (these need only earlier chunks' K/V, so their
exp-evictions overlap this chunk's K/V projection matmuls; their PSUM
accumulators and es_sum tiles stay open across it) -> K/V projection ->
diagonal attention blocks + head 3 -> partial output projection rows c.
Engine assignment: TensorE does all GEMMs; ScalarE does exp-evictions and
V-evictions plus a second HWDGE DMA ring for non-critical constants;
VectorE does rope, softmax row-sum accumulation (es_sum, fp16; one
ones-matmul per (c,h) turns it into l), normalization, and out-evictions;
GpSimd does the causal-strip mask multiplies and the 1/l
partition-broadcast (bf16).  Startup DMAs are split per-d-tile and
interleaved so the first matmul starts ~1us in; chunk-0 accumulation
groups are spread across the idle attention PSUM banks to track the DMA
pacing.  Causal score blocks are column-sliced (only tq >= tk columns
computed) with a single shared 128-wide triangular strip pattern for the
diagonal; out-projection PSUM shares the score pool's banks so the mm
pool stays free for the next chunk's projections.

Self-contained: shapes/sharding hardcoded for
  q_input/kv_input [2, 2048, 2048], 16 heads x 128 head_dim.
"""

import math

import numpy as np
import ml_dtypes

B, T, D, H = 2, 2048, 2048, 16
HD = 128          # head dim
HALF = HD // 2    # rope half
P = 128           # partitions
CHUNK = 512       # tq / free-dim chunk
NCORES = 8
GROUPS = 4        # head-groups (tensor-parallel degree per batch)
HPG = H // GROUPS # heads per group
GD = HPG * HD     # group width (512)
DT = D // P       # d-tiles (16)
TCH = T // CHUNK  # seq chunks (4)
TKT = T // P      # tk tiles (16)
CPT = CHUNK // P  # tk tiles per chunk (4)

TRACE = False       # set True before calling kernel() to capture an NTFF trace
LAST_RESULT = None  # BassKernelResults of the last kernel() call

_cache = {}


def _build_program(actions, npat1, npat2, repeat=1):
    """Build the per-core Bass program.

    actions: {(c, t): ("plain", lo) | ("strip", lo, idx) | ("wide", idx)}
    for every (tq-chunk, tk-tile) score block with >=1 unmasked element:
      - plain: columns [lo, CHUNK) fully unmasked, columns < lo fully masked
      - strip: like plain but columns [lo, lo+128) need pat128[idx]
      - wide:  full-width pattern pat512[idx]
    repeat: unroll the whole body N times (for differential timing in bench.py).
    """
    from contextlib import ExitStack

    import concourse.mybir as mybir
    import concourse.tile as tile
    from concourse import bacc
    from concourse.bass import ts

    fp32 = mybir.dt.float32
    fp16 = mybir.dt.float16
    bf16 = mybir.dt.bfloat16
    Copy = mybir.ActivationFunctionType.Copy
    Exp = mybir.ActivationFunctionType.Exp
    SCALE = 1.0 / math.sqrt(HD)

    nc = bacc.Bacc(
        "TRN2",
        target_bir_lowering=False,
        debug=False,
        enable_asserts=False,
        num_devices=NCORES,
    )

    xqT = nc.dram_tensor("xqT", [D, T], bf16, kind="ExternalInput").ap()
    xkvT = nc.dram_tensor("xkvT", [D, T], bf16, kind="ExternalInput").ap()
    wq = nc.dram_tensor("wq", [D, GD], bf16, kind="ExternalInput").ap()
    wk = nc.dram_tensor("wk", [D, GD], bf16, kind="ExternalInput").ap()
    wv = nc.dram_tensor("wv", [D, GD], bf16, kind="ExternalInput").ap()
    wo = nc.dram_tensor("wo", [GD, D], bf16, kind="ExternalInput").ap()
    # RoPE in head-dim-interleaved space (host permutes Wq/Wk columns so the
    # rope pair (j, j+64) lands on adjacent partitions (2j, 2j+1); scores are
    # invariant to a common Q/K head-dim permutation):
    #   rope'(x) = x * cs2 + swap_adjacent_pairs(x) * ss2
    # cs2[2j] = cs2[2j+1] = cos_j ; ss2[2j] = -sin_j, ss2[2j+1] = +sin_j
    cs2 = nc.dram_tensor("cs2", [P, T], bf16, kind="ExternalInput").ap()
    ss2 = nc.dram_tensor("ss2", [P, T], bf16, kind="ExternalInput").ap()
    pat1 = nc.dram_tensor("pat1", [npat1, P, P], bf16, kind="ExternalInput").ap()
    pat2 = nc.dram_tensor("pat2", [npat2, P, CHUNK], bf16, kind="ExternalInput").ap()
    out = nc.dram_tensor("out", [T, D], fp16, kind="ExternalOutput").ap()

    xkvr = xkvT.rearrange("(dt p) t -> p dt t", p=P)
    xqr = xqT.rearrange("(dt p) t -> p dt t", p=P)
    wqr = wq.rearrange("(dt p) n -> p dt n", p=P)
    wkr = wk.rearrange("(dt p) n -> p dt n", p=P)
    wvr = wv.rearrange("(dt p) n -> p dt n", p=P)
    wor = wo.rearrange("(h p) n -> p h n", p=P)

    tlists = {}
    for c in range(TCH):
        tlists[c] = sorted(t for (cc, t) in actions if cc == c)

    with ExitStack() as ctx:
        tc = ctx.enter_context(tile.TileContext(nc))
        const_pool = ctx.enter_context(tc.tile_pool(name="const", bufs=1))
        xpool = ctx.enter_context(tc.tile_pool(name="xchunk", bufs=2))
        qa_pool = ctx.enter_context(tc.tile_pool(name="qa", bufs=2))
        rope_pool = ctx.enter_context(tc.tile_pool(name="rope", bufs=2))
        exp_pool = ctx.enter_context(tc.tile_pool(name="exp", bufs=4))
        osb_pool = ctx.enter_context(tc.tile_pool(name="osb", bufs=4))
        lb_pool = ctx.enter_context(tc.tile_pool(name="lb", bufs=2))
        esum_pool = ctx.enter_context(tc.tile_pool(name="esum", bufs=4))
        mm_psum = ctx.enter_context(tc.tile_pool(name="mmps", bufs=2, space="PSUM"))
        s_psum = ctx.enter_context(tc.tile_pool(name="sps", bufs=3, space="PSUM"))
        o_psum = ctx.enter_context(tc.tile_pool(name="ops", bufs=2, space="PSUM"))
        l_psum = ctx.enter_context(tc.tile_pool(name="lps", bufs=1, space="PSUM"))

        # persistent SBUF tensors
        wq_sb = const_pool.tile([P, DT, GD], bf16, tag="wq")
        wk_sb = const_pool.tile([P, DT, GD], bf16, tag="wk")
        wv_sb = const_pool.tile([P, DT, GD], bf16, tag="wv")
        wo_sb = const_pool.tile([P, HPG, D], bf16, tag="wo")
        cs2_sb = const_pool.tile([P, T], bf16, tag="cs2")
        ss2_sb = const_pool.tile([P, T], bf16, tag="ss2")
        use_wide = any(a[0] == "wide" for a in actions.values())
        pat1_sb = const_pool.tile([P, npat1, P], bf16, tag="pat1")
        pat2_sb = (
            const_pool.tile([P, npat2, CHUNK], bf16, tag="pat2") if use_wide else None
        )
        ones_sb = const_pool.tile([P, 1], bf16, tag="ones")
        KT = const_pool.tile([P, HPG, T], bf16, tag="KT")
        V = const_pool.tile([P, TKT, GD], bf16, tag="V")

        nc.vector.memset(ones_sb[:], 1.0)

        SHUF_MASK = [i + 1 - 2 * (i % 2) for i in range(32)]  # [1,0,3,2,...]

        def rope_evict(ps, c, dest):
            # ps: PSUM [P, CHUNK] fp32, partitions = interleaved head_dim
            # dest = ps * cs2[chunk] + swap_adjacent_pairs(ps) * ss2[chunk]
            rsw = rope_pool.tile([P, CHUNK], fp32, tag="rsw")
            nc.vector.stream_shuffle(rsw[:], ps[:], SHUF_MASK)
            nc.vector.tensor_mul(dest, ps[:], cs2_sb[:, ts(c, CHUNK)])
            t2 = rope_pool.tile([P, CHUNK], bf16, tag="t2")
            nc.vector.tensor_mul(t2[:], rsw[:], ss2_sb[:, ts(c, CHUNK)])
            nc.vector.tensor_add(dest, dest, t2[:])

        for _rep in range(repeat):
            # ---- startup: interleave per-d-tile weight/x DMAs so the first
            # projection matmul only waits for one 128x512 slice of each.
            # second DMA ring (Activation HWDGE): constants that aren't on the
            # critical startup path, in need-by order
            nc.scalar.dma_start(cs2_sb[:], cs2)
            nc.scalar.dma_start(ss2_sb[:], ss2)
            nc.scalar.dma_start(wq_sb[:], wqr)
            nc.scalar.dma_start(pat1_sb[:], pat1.rearrange("j p n -> p j n"))
            if use_wide:
                nc.scalar.dma_start(pat2_sb[:], pat2.rearrange("j p n -> p j n"))
            nc.scalar.dma_start(wo_sb[:], wor)
            # primary ring: per-d-tile triples pacing the chunk-0 projections
            xk0 = xpool.tile([P, DT, CHUNK], bf16, tag="xk")
            for d in range(DT):
                nc.sync.dma_start(wk_sb[:, d, :], wkr[:, d, :])
                nc.sync.dma_start(xk0[:, d, :], xkvr[:, d, ts(0, CHUNK)])
                nc.sync.dma_start(wv_sb[:, d, :], wvr[:, d, :])
            xq0 = xpool.tile([P, DT, CHUNK], bf16, tag="xq")
            for d in range(DT):
                nc.sync.dma_start(xq0[:, d, :], xqr[:, d, ts(0, CHUNK)])

            def attn_blocks(c, h, QT, opst, es_sum, blocks, started, do_stop):
                """Score/exp/AV for the given tk tiles of (c, h); returns
                whether the opst/es_sum accumulation has been started."""
                for j, t in enumerate(blocks):
                    act = actions[(c, t)]
                    if act[0] == "wide":
                        lo = 0
                    else:
                        lo = act[1]
                    n = CHUNK - lo
                    spst = s_psum.tile([P, CHUNK], fp32, tag="s")
                    nc.tensor.matmul(
                        spst[:, lo:], KT[:, h, ts(t, P)],
                        QT[:, h, lo:], start=True, stop=True,
                    )
                    es = exp_pool.tile([P, CHUNK], bf16, tag="es")
                    nc.scalar.activation(es[:, lo:], spst[:, lo:], Exp, scale=SCALE)
                    if act[0] == "strip":
                        w = min(P, n)
                        nc.gpsimd.tensor_mul(
                            es[:, lo:lo + w], es[:, lo:lo + w],
                            pat1_sb[:, act[2], :w],
                        )
                    elif act[0] == "wide":
                        nc.gpsimd.tensor_mul(es[:], es[:], pat2_sb[:, act[1], :])
                    first = not started
                    last = do_stop and (j == len(blocks) - 1)
                    assert not first or lo == 0
                    if first:
                        nc.vector.tensor_copy(es_sum[:], es[:])
                    else:
                        nc.vector.tensor_add(
                            es_sum[:, lo:], es_sum[:, lo:], es[:, lo:]
                        )
                    nc.tensor.matmul(
                        opst[:, lo:], V[:, t, ts(h, HD)], es[:, lo:],
                        start=first, stop=last,
                    )
                    started = True
                return started

            def attn_finalize(h, AT, opst, es_sum):
                # row sums l = ones^T @ es_sum, then AT = opst / l
                lpst = l_psum.tile([1, CHUNK], fp32, tag="l")
                nc.tensor.matmul(
                    lpst[:], ones_sb[:], es_sum[:], start=True, stop=True
                )
                rec = lb_pool.tile([1, CHUNK], bf16, tag="rec")
                with nc.allow_low_precision(reason="1/l in bf16: 0.4% noise vs 2e-2 gate"):
                    nc.vector.reciprocal(rec[:], lpst[:])
                # broadcast 1/l across partitions on GpSimd
                lbs = lb_pool.tile([P, CHUNK], bf16, tag="lbs")
                nc.gpsimd.partition_broadcast(lbs[:], rec[:])
                nc.vector.tensor_mul(AT[:, h, :], opst[:], lbs[:])

            xk_next, xq_next = xk0, xq0
            pending_out = None
            for c in range(TCH):
                xk, xq = xk_next, xq_next
                tlist = tlists[c]
                # during the DMA-paced chunk 0, spread accumulation groups
                # over the idle attention PSUM banks so more matmuls are
                # ready per arriving d-slice
                def kproj_ps(h):
                    if c == 0 and h >= 2:
                        ps = s_psum.tile([P, CHUNK], fp32, tag="s")
                    else:
                        ps = mm_psum.tile([P, CHUNK], fp32, tag="mm")
                    return ps

                def vproj_ps(s):
                    if c == 0 and s < 2:
                        ps = o_psum.tile([P, CHUNK], fp32, tag="o")
                    elif c == 0 and s == 2:
                        ps = s_psum.tile([P, CHUNK], fp32, tag="s")
                    else:
                        ps = mm_psum.tile([P, GD], fp32, tag="mm")
                    return ps

                def kv_proj():
                    for h in range(HPG):
                        ps = kproj_ps(h)
                        for d in range(DT):
                            nc.tensor.matmul(
                                ps[:], wk_sb[:, d, ts(h, HD)], xk[:, d, :],
                                start=(d == 0), stop=(d == DT - 1),
                            )
                        rope_evict(ps, c, KT[:, h, ts(c, CHUNK)])
                    for s in range(CPT):
                        ps = vproj_ps(s)
                        for d in range(DT):
                            nc.tensor.matmul(
                                ps[:], xk[:, d, ts(s, P)], wv_sb[:, d, :],
                                start=(d == 0), stop=(d == DT - 1),
                            )
                        nc.scalar.activation(V[:, c * CPT + s, :], ps[:], Copy)

                def q_proj(QT):
                    for h in range(HPG):
                        ps = kproj_ps(h)
                        for d in range(DT):
                            nc.tensor.matmul(
                                ps[:], wq_sb[:, d, ts(h, HD)], xq[:, d, :],
                                start=(d == 0), stop=(d == DT - 1),
                            )
                        rope_evict(ps, c, QT[:, h, :])

                def prefetch_next():
                    nonlocal xk_next, xq_next
                    if c + 1 < TCH:
                        xk_next = xpool.tile([P, DT, CHUNK], bf16, tag="xk")
                        nc.sync.dma_start(xk_next[:], xkvr[:, :, ts(c + 1, CHUNK)])
                        xq_next = xpool.tile([P, DT, CHUNK], bf16, tag="xq")
                        nc.sync.dma_start(xq_next[:], xqr[:, :, ts(c + 1, CHUNK)])

                QT = qa_pool.tile([P, HPG, CHUNK], bf16, tag="QT")
                AT = qa_pool.tile([P, HPG, CHUNK], bf16, tag="AT")
                if c == 0:
                    # chunk 0 is DMA-paced and has no off-diagonal blocks:
                    # original order (KV -> Q -> attention)
                    kv_proj()
                    q_proj(QT)
                    prefetch_next()
                    for h in range(HPG):
                        opst = o_psum.tile([P, CHUNK], fp32, tag="o")
                        es_sum = esum_pool.tile([P, CHUNK], fp16, tag="es_sum")
                        attn_blocks(c, h, QT, opst, es_sum, tlist, False, True)
                        attn_finalize(h, AT, opst, es_sum)
                else:
                    # off-diagonal attention blocks only need KT/V of earlier
                    # chunks: run heads 0-1's off-diagonal part BEFORE this
                    # chunk's K/V projection (their exp-evictions overlap the
                    # projection matmuls), holding opst/es_sum open across it
                    part1 = [t for t in tlist if t < c * CPT]
                    part2 = [t for t in tlist if t >= c * CPT]
                    q_proj(QT)
                    held = {}
                    for h in (0, 1, 2):
                        if h < 2:
                            opst = o_psum.tile([P, CHUNK], fp32, tag="o")
                        else:
                            # park h2's accumulator in a score-pool bank (the
                            # s pool is idle during this chunk's projections)
                            opst = s_psum.tile([P, CHUNK], fp32, tag="s")
                        es_sum = esum_pool.tile([P, CHUNK], fp16, tag="es_sum")
                        started = attn_blocks(
                            c, h, QT, opst, es_sum, part1, False, False
                        )
                        held[h] = (opst, es_sum, started)
                    kv_proj()
                    prefetch_next()
                    for h in (0, 1, 2):
                        opst, es_sum, started = held[h]
                        attn_blocks(c, h, QT, opst, es_sum, part2, started, True)
                        attn_finalize(h, AT, opst, es_sum)
                    for h in (3,):
                        opst = o_psum.tile([P, CHUNK], fp32, tag="o")
                        es_sum = esum_pool.tile([P, CHUNK], fp16, tag="es_sum")
                        attn_blocks(c, h, QT, opst, es_sum, tlist, False, True)
                        attn_finalize(h, AT, opst, es_sum)

                # ---- partial output projection, DEFERRED one chunk:
                # emitting chunk c-1's out-projection here (after chunk c's
                # attention in program order = lower scheduler priority) gives
                # the PE ready filler work during c's eviction-paced
                # attention tail.  out[tq, :] = sum_h attn_h^T.T @ Wo_h
                def out_proj(cc, ATx):
                    for m in range(CPT):
                        for oc in range(D // CHUNK):
                            # mm pool is idle between this chunk's KV
                            # projection and the next chunk's Q projection
                            ps = mm_psum.tile([P, CHUNK], fp32, tag="mm")
                            for h in range(HPG):
                                nc.tensor.matmul(
                                    ps[:], ATx[:, h, ts(m, P)],
                                    wo_sb[:, h, ts(oc, CHUNK)],
                                    start=(h == 0), stop=(h == HPG - 1),
                                )
                            ob = osb_pool.tile([P, CHUNK], fp16, tag="ob")
                            nc.vector.tensor_copy(ob[:], ps[:])
                            nc.sync.dma_start(
                                out[ts(cc * CPT + m, P), ts(oc, CHUNK)], ob[:]
                            )

                if pending_out is not None:
                    out_proj(*pending_out)
                pending_out = (c, AT)
            out_proj(*pending_out)

    nc.compile()
    return nc


def _interleave_heads(W):
    """Permute each 128-wide head block of columns: new[2j]=old[j], new[2j+1]=old[64+j]."""
    d, gd = W.shape
    return np.ascontiguousarray(
        W.reshape(d, gd // HD, 2, HALF).transpose(0, 1, 3, 2).reshape(d, gd)
    )


def _rope_tables(cos, sin):
    """cs2[2j]=cs2[2j+1]=cos_j ; ss2[2j]=-sin_j, ss2[2j+1]=+sin_j  (both [128, T])."""
    bf = ml_dtypes.bfloat16
    cosT = np.ascontiguousarray(cos.T)  # [HALF, T]
    sinT = np.ascontiguousarray(sin.T)
    cs2 = np.repeat(cosT, 2, axis=0).astype(bf)
    ss2 = np.stack([-sinT, sinT], axis=1).reshape(HD, -1).astype(bf)
    return cs2, ss2


def _mask_actions(mask):
    """Classify every [CHUNK tq x P tk] score block of the mask.

    Returns (actions, pat128 [npat1,P,P], pat512 [npat2,P,CHUNK]); see
    _build_program for the action encoding.  Patterns are stored transposed
    ([tk, tq]) to match the score layout.  Blocks with no unmasked element
    are omitted (skipped entirely).
    """
    bf = ml_dtypes.bfloat16
    m = np.asarray(mask).reshape(T, T).astype(bool)
    actions = {}
    pats1, pat1_keys = [], {}
    pats2, pat2_keys = [], {}

    def wide(bt):
        key = bt.tobytes()
        if key not in pat2_keys:
            pat2_keys[key] = len(pats2)
            pats2.append(bt.astype(bf))
        return ("wide", pat2_keys[key])

    for c in range(TCH):
        first_in_row = True
        for t in range(TKT):
            blk = m[c * CHUNK:(c + 1) * CHUNK, t * P:(t + 1) * P]
            if not blk.any():
                continue
            bt = np.ascontiguousarray(blk.T)  # [tk, tq]
            colact = bt.any(axis=0)
            lo = int(np.argmax(colact))
            if not colact[lo:].all() or (first_in_row and lo > 0):
                actions[(c, t)] = wide(bt)
            else:
                w = min(P, CHUNK - lo)
                strip = bt[:, lo:lo + w]
                rest = bt[:, lo + w:]
                if not rest.all():
                    actions[(c, t)] = wide(bt)
                elif strip.all():
                    actions[(c, t)] = ("plain", lo)
                else:
                    sp = np.ones((P, P), bf)
                    sp[:, :w] = strip.astype(bf)
                    key = sp.tobytes()
                    if key not in pat1_keys:
                        pat1_keys[key] = len(pats1)
                        pats1.append(sp)
                    actions[(c, t)] = ("strip", lo, pat1_keys[key])
            first_in_row = False
    if not pats1:
        pats1.append(np.zeros((P, P), bf))
    if not pats2:
        pats2.append(np.zeros((P, CHUNK), bf))
    return actions, np.ascontiguousarray(np.stack(pats1)), np.ascontiguousarray(np.stack(pats2))


def kernel(**inputs):
    global LAST_RESULT
    q_input = np.asarray(inputs["q_input"], dtype=np.float32)
    kv_input = np.asarray(inputs["kv_input"], dtype=np.float32)
    cos = np.asarray(inputs["cos"], dtype=np.float32)
    sin = np.asarray(inputs["sin"], dtype=np.float32)
    Wq = np.asarray(inputs["Wq"], dtype=np.float32)
    Wk = np.asarray(inputs["Wk"], dtype=np.float32)
    Wv = np.asarray(inputs["Wv"], dtype=np.float32)
    Wo = np.asarray(inputs["Wo"], dtype=np.float32)

    actions, pats1, pats2 = _mask_actions(inputs["mask"])
    key = (tuple(sorted(actions.items())), pats1.shape[0], pats2.shape[0])
    if key not in _cache:
        _cache[key] = _build_program(
            actions, int(pats1.shape[0]), int(pats2.shape[0])
        )
    nc = _cache[key]

    bf = ml_dtypes.bfloat16
    cs2, ss2 = _rope_tables(cos, sin)
    xq = [np.ascontiguousarray(q_input[b].T).astype(bf) for b in range(B)]
    xkv = [np.ascontiguousarray(kv_input[b].T).astype(bf) for b in range(B)]
    wq_g = [_interleave_heads(Wq[:, g * GD:(g + 1) * GD]).astype(bf) for g in range(GROUPS)]
    wk_g = [_interleave_heads(Wk[:, g * GD:(g + 1) * GD]).astype(bf) for g in range(GROUPS)]
    wv_g = [np.ascontiguousarray(Wv[:, g * GD:(g + 1) * GD]).astype(bf) for g in range(GROUPS)]
    wo_g = [np.ascontiguousarray(Wo[g * GD:(g + 1) * GD, :]).astype(bf) for g in range(GROUPS)]

    in_maps = []
    for core in range(NCORES):
        b, g = divmod(core, GROUPS)
        in_maps.append({
            "xqT": xq[b],
            "xkvT": xkv[b],
            "wq": wq_g[g],
            "wk": wk_g[g],
            "wv": wv_g[g],
            "wo": wo_g[g],
            "cs2": cs2,
            "ss2": ss2,
            "pat1": pats1,
            "pat2": pats2,
        })

    from concourse import bass_utils

    res = bass_utils.run_bass_kernel_spmd(
        nc, in_maps, core_ids=list(range(NCORES)), trace=TRACE
    )
    LAST_RESULT = res
    outs = [r["out"] for r in res.results]
    full = np.stack([
        sum(outs[b * GROUPS + g].astype(np.float32) for g in range(GROUPS))
        for b in range(B)
    ])
    return np.ascontiguousarray(full)
